# revision 5
# baseline (speedup 1.0000x reference)
"""Trainium2 Bass kernel for nn_HCAProtoNet (vq_codebook).

Data-parallel over 8 NeuronCores: each core processes 8192 rows of x.
Per core the pipeline is:
  DMA x tile -> PE transpose (f32) -> ACT copy PSUM->SBUF -> PE matmul
  against a fused prototype matrix (35 metric columns) -> PE transpose
  back to row-major -> DVE/ACT postprocessing (norms, softmax, entropy,
  rare-class gating) -> DMA out.
`features` output is the identity of x and is returned host-side.
"""

import numpy as np

B, D = 65536, 256
N_CORES = 8
BL = B // N_CORES          # rows per core
NCLS = 5
KS, KR = 20, 5
RARE = (0, 1)
TEMP = 1.5
EPS = 1e-8
NM = 35                    # metric cols: 20 shared | 5 ls | 5 rare0 | 5 rare1
TPG = 4                    # 128-row subtiles per matmul group
GR = 128 * TPG             # 512 rows per group
NG = BL // GR              # 16 groups
PP = 4                     # matmul groups per postproc batch
NB = NG // PP              # postproc batches
J = PP * TPG               # subtiles per postproc batch (16)
RW = 64                    # padded metric pitch in the row-major PSUM tile

_prog = None


def _build_program():
    import concourse.bacc as bacc
    import concourse.tile as tile
    import concourse.mybir as mybir

    dt = mybir.dt
    f32 = dt.float32
    Alu = mybir.AluOpType
    Act = mybir.ActivationFunctionType
    Ax = mybir.AxisListType

    nc = bacc.Bacc(
        "TRN2",
        target_bir_lowering=False,
        debug=False,
        enable_asserts=False,
        num_devices=N_CORES,
    )

    x_d = nc.dram_tensor("x", [BL, D], f32, kind="ExternalInput").ap()
    f16 = dt.float16
    m0_d = nc.dram_tensor("m0", [128, NM], f16, kind="ExternalInput").ap()
    m1_d = nc.dram_tensor("m1", [128, NM], f16, kind="ExternalInput").ap()
    id_d = nc.dram_tensor("ident", [128, 128], f32, kind="ExternalInput").ap()
    g_d = nc.dram_tensor("gates", [128, 3], f32, kind="ExternalInput").ap()
    lg_d = nc.dram_tensor("logits", [BL, NCLS], f32, kind="ExternalOutput").ap()
    sh_d = nc.dram_tensor("shared_sim", [BL, KS], f32, kind="ExternalOutput").ap()
    lr_d = nc.dram_tensor("logits_rare", [BL, NCLS], f32, kind="ExternalOutput").ap()

    with tile.TileContext(nc) as tc, \
            tc.tile_pool(name="const", bufs=1) as constp, \
            tc.tile_pool(name="xin", bufs=3) as xp, \
            tc.tile_pool(name="xts", bufs=4) as xtp, \
            tc.tile_pool(name="sqs", bufs=2) as sqp, \
            tc.tile_pool(name="stat", bufs=2) as stp, \
            tc.tile_pool(name="outp", bufs=2) as op_, \
            tc.tile_pool(name="ps", bufs=1, space="PSUM") as pp_:

        m0_sb = constp.tile([128, NM], f16)
        nc.sync.dma_start(m0_sb[:], m0_d)
        m1_sb = constp.tile([128, NM], f16)
        nc.sync.dma_start(m1_sb[:], m1_d)
        id_sb = constp.tile([128, 128], f32)
        nc.sync.dma_start(id_sb[:], id_d)
        g_sb = constp.tile([128, 3], f32)
        nc.sync.dma_start(g_sb[:], g_d)

        for pb in range(NB):
            rrow = pp_.tile([128, J, RW], f32, tag="rrow", bufs=2, name=f"rrow{pb}")
            n2 = stp.tile([128, J], f32, tag="n2", name=f"n2_{pb}")
            for gi in range(PP):
                g = pb * PP + gi
                xnat = xp.tile([128, TPG, D], f32, tag="x", name=f"x{g}")
                nc.sync.dma_start(
                    xnat[:],
                    x_d[g * GR:(g + 1) * GR].rearrange("(t p) d -> p t d", p=128),
                )
                xt = []
                for c in range(2):
                    tps = pp_.tile([128, 512], f32, tag="xT", bufs=3, name=f"xt{g}_{c}")
                    for t in range(TPG):
                        nc.tensor.transpose(
                            tps[:, t * 128:(t + 1) * 128],
                            xnat[:, t, c * 128:(c + 1) * 128],
                            id_sb[:],
                        )
                    xt.append(tps)
                xts = []
                for c in range(2):
                    s = xtp.tile([128, 512], f16, tag="xts", name=f"xts{g}_{c}")
                    nc.vector.tensor_copy(s[:], xt[c][:])
                    xts.append(s)
                for t in range(TPG):
                    ji = gi * TPG + t
                    nc.tensor.matmul(rrow[:, ji, 0:NM],
                                     xts[0][:, t * 128:(t + 1) * 128],
                                     m0_sb[:], start=True, stop=False)
                    nc.tensor.matmul(rrow[:, ji, 0:NM],
                                     xts[1][:, t * 128:(t + 1) * 128],
                                     m1_sb[:], start=False, stop=True)
                    sq = sqp.tile([128, D], f32, tag="sq", name=f"sq{g}_{t}")
                    nc.scalar.activation(sq[:], xnat[:, t, :], Act.Square,
                                         accum_out=n2[:, ji:ji + 1])

            # ---------- postprocessing over J=16 subtiles (2048 rows) ----------
            rn = stp.tile([128, J], f32, tag="rn", name=f"rn{pb}")
            nc.vector.reciprocal(rn[:], n2[:])
            inv = stp.tile([128, J], f32, tag="inv", name=f"inv{pb}")
            nc.scalar.activation(inv[:], rn[:], Act.Sqrt)

            shls = op_.tile([128, J, 25], f32, tag="shls", name=f"shls{pb}")
            nc.vector.tensor_tensor(
                shls[:], rrow[:, :, 0:25], inv[:].broadcast_to([128, J, 25]),
                Alu.mult,
            )
            mx0 = stp.tile([128, J], f32, tag="mx0", name=f"mx0_{pb}")
            nc.vector.tensor_reduce(mx0[:], rrow[:, :, 25:30], axis=Ax.X, op=Alu.max)
            mx1 = stp.tile([128, J], f32, tag="mx1", name=f"mx1_{pb}")
            nc.vector.tensor_reduce(mx1[:], rrow[:, :, 30:35], axis=Ax.X, op=Alu.max)

            e = op_.tile([128, J, NCLS], f32, tag="e", name=f"e{pb}")
            nc.scalar.activation(e[:], shls[:, :, 20:25], Act.Exp, scale=1.0 / TEMP)
            S = stp.tile([128, J], f32, tag="S", name=f"S{pb}")
            nc.vector.tensor_reduce(S[:], e[:], axis=Ax.X, op=Alu.add)
            rS = stp.tile([128, J], f32, tag="rS", name=f"rS{pb}")
            nc.vector.reciprocal(rS[:], S[:])
            p = op_.tile([128, J, NCLS], f32, tag="p", name=f"p{pb}")
            nc.vector.tensor_tensor(
                p[:], e[:], rS[:].broadcast_to([128, J, NCLS]), Alu.mult,
            )
            lq = op_.tile([128, J, NCLS], f32, tag="lq", name=f"lq{pb}")
            nc.scalar.activation(lq[:], p[:], Act.Ln, bias=g_sb[:, 2:3])
            plp = op_.tile([128, J, NCLS], f32, tag="plp", name=f"plp{pb}")
            nc.vector.tensor_tensor(plp[:], p[:], lq[:], Alu.mult)
            spl = stp.tile([128, J], f32, tag="spl", name=f"spl{pb}")
            nc.vector.tensor_reduce(spl[:], plp[:], axis=Ax.X, op=Alu.add)
            u = stp.tile([128, J], f32, tag="u", name=f"u{pb}")
            nc.vector.tensor_tensor(u[:], spl[:], inv[:], Alu.mult)

            lrare = op_.tile([128, J, NCLS], f32, tag="lrare", name=f"lrare{pb}")
            nc.gpsimd.memset(lrare[:], 0.0)
            for c, mx in ((0, mx0), (1, mx1)):
                v = stp.tile([128, J], f32, tag=f"v{c}", name=f"v{c}_{pb}")
                nc.vector.tensor_tensor(v[:], mx[:], u[:], Alu.mult)
                lrc = stp.tile([128, J], f32, tag=f"lrc{c}", name=f"lrc{c}_{pb}")
                nc.vector.tensor_scalar(
                    lrc[:], v[:], g_sb[:, c:c + 1], None, Alu.mult,
                )
                nc.vector.tensor_copy(lrare[:, :, c], lrc[:])
            lgt = op_.tile([128, J, NCLS], f32, tag="lgt", name=f"lgt{pb}")
            nc.vector.tensor_tensor(lgt[:], shls[:, :, 20:25], lrare[:], Alu.add)

            rows = slice(pb * J * 128, (pb + 1) * J * 128)
            nc.sync.dma_start(
                sh_d[rows].rearrange("(t p) k -> p t k", p=128), shls[:, :, 0:20],
            )
            nc.sync.dma_start(
                lg_d[rows].rearrange("(t p) k -> p t k", p=128), lgt[:],
            )
            nc.sync.dma_start(
                lr_d[rows].rearrange("(t p) k -> p t k", p=128), lrare[:],
            )

    nc.compile()
    return nc


def _prototype_matrix(sp, W, rp):
    def l2n(v):
        n = np.linalg.norm(v, axis=-1, keepdims=True)
        return v / np.maximum(n, 1e-12)

    spn = l2n(sp.astype(np.float32))
    rpn = l2n(rp.astype(np.float32))
    M = np.zeros((D, NM), np.float32)
    M[:, 0:20] = spn.T
    M[:, 20:25] = spn.T @ W.astype(np.float32)
    M[:, 25:30] = rpn[0].T
    M[:, 30:35] = rpn[1].T
    return M


def _run_traced(in_maps, tmpdir):
    global _prog
    if _prog is None:
        _prog = _build_program()
    from concourse.bass_utils import run_bass_kernel_spmd

    return run_bass_kernel_spmd(_prog, in_maps, list(range(N_CORES)),
                                trace=True, tmpdir=tmpdir)


def _run(in_maps, trace=False):
    global _prog
    if _prog is None:
        _prog = _build_program()
    from concourse.bass_utils import run_bass_kernel_spmd

    return run_bass_kernel_spmd(_prog, in_maps, list(range(N_CORES)), trace=trace)


def _make_in_maps(x, shared_prototypes, W_shared_to_class, rare_prototypes,
                  rarity_factor):
    x = np.ascontiguousarray(np.asarray(x, dtype=np.float32))
    M = _prototype_matrix(
        np.asarray(shared_prototypes), np.asarray(W_shared_to_class),
        np.asarray(rare_prototypes),
    )
    rf = np.asarray(rarity_factor, dtype=np.float32)
    gv = -(rf[list(RARE)] / np.float32(np.log(5.0))).astype(np.float32)
    gates = np.tile(np.array([gv[0], gv[1], EPS], np.float32)[None, :], (128, 1))
    ident = np.eye(128, dtype=np.float32)
    m0 = np.ascontiguousarray(M[0:128]).astype(np.float16)
    m1 = np.ascontiguousarray(M[128:256]).astype(np.float16)
    in_maps = [
        {
            "x": x[i * BL:(i + 1) * BL],
            "m0": m0,
            "m1": m1,
            "ident": ident,
            "gates": gates,
        }
        for i in range(N_CORES)
    ]
    return x, in_maps


def kernel(x, shared_prototypes, W_shared_to_class, rare_prototypes,
           rarity_factor):
    x, in_maps = _make_in_maps(
        x, shared_prototypes, W_shared_to_class, rare_prototypes, rarity_factor,
    )
    res = _run(in_maps).results
    logits = np.concatenate([r["logits"] for r in res], axis=0)
    shared = np.concatenate([r["shared_sim"] for r in res], axis=0)
    lrare = np.concatenate([r["logits_rare"] for r in res], axis=0)
    return (logits, x, shared, lrare)
